# revision 12
# baseline (speedup 1.0000x reference)
"""Causal self-attention (B=4, T=2048, C=1024, H=16, D=64) on 8 trn2 NeuronCores.

Sharding: core = 2*b + g  (b = batch 0..3, g = head-group 0..1, 8 heads each).
Data parallel over B, tensor parallel over heads; each core computes a partial
out-projection (its 512 y-channels x out_w columns) and the host sums the two
partials per batch (the tensor-parallel all-reduce) and adds biases.

v3c schedule: fine-grained weave; xT input DMA split over two queues;
po2 evacuation on ScalarE (off the DVE PSUM-release path; normalize muls
stay on DVE so the GpSimd tri stream is undisturbed).  Attention chunks run in order 0,1,2,3 (so
the first exp is ~13us in, right after wave0), and ALL remaining projection /
out-projection matmuls are pulled as single-matmul fillers INSIDE the
attention ki loop, between S(ki) and PV(ki-1).  Each weave gets 96 filler
matmuls:
  wave0:    qk t4=0 ki-major across all 8 PSUM banks (xT-DMA-paced), v 0-3.
  weave(0): 16 iters, fillers = wave1  (qk t4=1 + v 4-7)
  weave(1): 32 iters, fillers = wave2
  weave(2): 48 iters, fillers = wave3
  weave(3): 64 iters, fillers = out-proj chunks 0,1,2
  tail:     out-proj chunk 3.
The ScalarE exp stream paces the late weaves (~1.15us/iter) while the PE
stream (S pair row-tiled concurrent + PV pair + fillers) stays just under it;
early weaves are PE-bound because wave(qc+1) must land before weave(qc+1).

Numerics (unchanged from v2):
  - k-projection bias is dropped: softmax over k is invariant to the
    per-query constant q.bk (exact identity, not an approximation).
  - exp without max subtraction (|S| < ~5, exact-safe in f32/bf16).
  - Both heads of a pair share one 2-bank PSUM score tile; ONE activation
    instruction exps both (strided 2-window AP), halving ACT instr overhead.
  - l = sum(P) rides the PV matmul as a ones column; the whole [O;l] block
    (65 partitions x both heads) leaves PSUM in ONE DVE copy, and one
    spread/recip/gather/broadcast chain normalizes both heads.
"""

import os
import numpy as np
import ml_dtypes
from contextlib import ExitStack

B, T, C, H, D = 4, 2048, 1024, 16, 64
P = 128
N_CORES = 8
HPG = H // 2          # heads per group/core = 8
GC = HPG * D          # channels per group = 512
BF16 = ml_dtypes.bfloat16

_BUILT = {}
_TRI = "gpsimd"          # engine for the diagonal tri mask
_TRI_ENGINE = lambda nc: getattr(nc, _TRI)
_CHAIN_DMA_ENGINE = "sync"    # queue for the small normalize-chain DMAs
_CHAIN_DMA = lambda nc: getattr(nc, _CHAIN_DMA_ENGINE)
_STORE_DMA_ENGINE = "sync"  # queue for the out stores + yT shifts
_STORE_DMA = lambda nc: getattr(nc, _STORE_DMA_ENGINE)
_ABLATE_EXP = False   # timing ablation only: 1-col exps (breaks numerics)


def _split_multiwait_sync(nc):
    """This container's walrus rejects instructions carrying more than one
    sync-wait command ("Too many sync wait commands", setupSyncWait). Tile's
    scheduler emits such instructions (e.g. the end-of-context drain waits on
    every DMA-queue semaphore at once). Split them: hoist all but the last
    wait onto single-wait Drain instructions inserted just before, on the
    same engine — semantically identical (engine stalls on each in turn)."""
    import bass_rust
    from concourse import mybir

    n = 0
    for func in nc.m.functions:
        for block in func.blocks:
            insts = list(block.instructions)
            out = []
            changed = False
            for inst in insts:
                si = inst.sync_info
                waits = list(si.on_wait) if si is not None and si.on_wait else []
                if len(waits) > 1:
                    changed = True
                    for w in waits[:-1]:
                        d = mybir.InstDrain(
                            name=f"{inst.name}_swait{n}", ins=[], outs=[])
                        n += 1
                        d.engine = inst.engine
                        d.sync_info = bass_rust.SyncInfo(
                            on_wait=[w], on_update=[])
                        out.append(d)
                    si.on_wait = [waits[-1]]
                    inst.sync_info = si
                out.append(inst)
            if changed:
                block.instructions = out


def _build_bass(reps=1):
    """Build the (core-uniform) Bass program once per process.

    reps > 1 emits the whole body N times inside one NEFF — used only by the
    timing harness to amortize the multi-ms per-dispatch overhead of this
    axon client (NTFF profiling is unavailable here)."""
    key = ("nc", reps)
    if key in _BUILT:
        return _BUILT[key]

    import concourse.bass as bass
    import concourse.tile as tile
    from concourse import mybir

    DT = mybir.dt.bfloat16
    F32 = mybir.dt.float32

    nc = bass.Bass("TRN2", target_bir_lowering=False, debug=False)

    xT_d = nc.dram_tensor("xT", [P, 8, T], DT, kind="ExternalInput").ap()
    wqk_d = nc.dram_tensor("wqk", [P, 8, 1024], DT, kind="ExternalInput").ap()
    wv_d = nc.dram_tensor("wv", [P, 8, GC], DT, kind="ExternalInput").ap()
    bqk_d = nc.dram_tensor("bqk", [P, 8], F32, kind="ExternalInput").ap()
    wo_d = nc.dram_tensor("wo", [P, 4, 1024], DT, kind="ExternalInput").ap()
    tri_d = nc.dram_tensor("tri", [P, P], DT, kind="ExternalInput").ap()
    out_d = nc.dram_tensor("out", [T, 1024], F32, kind="ExternalOutput").ap()

    with tile.TileContext(nc) as tc, ExitStack() as ctx:
        consts = ctx.enter_context(tc.tile_pool(name="consts", bufs=1))
        ppool = ctx.enter_context(tc.tile_pool(name="ppool", bufs=6))
        npool = ctx.enter_context(tc.tile_pool(name="npool", bufs=5))
        outp = ctx.enter_context(tc.tile_pool(name="outp", bufs=4))
        # PSUM: 2x1-bank (mm) + 2x2-bank (s2) + 1x2-bank (po2) = 8 banks
        mmps = ctx.enter_context(
            tc.tile_pool(name="mmps", bufs=2, space="PSUM"))
        sps = ctx.enter_context(
            tc.tile_pool(name="sps", bufs=2, space="PSUM"))
        ops = ctx.enter_context(
            tc.tile_pool(name="ops", bufs=1, space="PSUM"))

        for rep in range(reps):
            _emit_body(nc, tc, consts, ppool, npool, outp, mmps, sps, ops,
                       xT_d, wqk_d, wv_d, bqk_d, wo_d, tri_d, out_d, rep)

    _split_multiwait_sync(nc)
    _BUILT[key] = nc
    return nc


def _emit_body(nc, tc, consts, ppool, npool, outp, mmps, sps, ops,
               xT_d, wqk_d, wv_d, bqk_d, wo_d, tri_d, out_d, rep):
    from concourse import mybir
    DT = mybir.dt.bfloat16
    F32 = mybir.dt.float32
    EXP = mybir.ActivationFunctionType.Exp
    MUL = mybir.AluOpType.mult

    # --- persistent SBUF tensors (same tag across reps -> shared slots) ---
    xT = consts.tile([P, 8, T], DT, tag="xT", name=f"xT{rep}")
    wqk = consts.tile([P, 8, 1024], DT, tag="wqk", name=f"wqk{rep}")
    wv = consts.tile([P, 8, GC], DT, tag="wv", name=f"wv{rep}")
    bqk = consts.tile([P, 8], F32, tag="bqk", name=f"bqk{rep}")
    wo = consts.tile([P, 4, 1024], DT, tag="wo", name=f"wo{rep}")
    tri = consts.tile([P, P], DT, tag="tri", name=f"tri{rep}")
    qkT = consts.tile([P, 8, T], DT, tag="qkT", name=f"qkT{rep}")
    v = consts.tile([P, 16, HPG, 65], DT, tag="v", name=f"v{rep}")
    yT = consts.tile([P, 4, T], DT, tag="yT", name=f"yT{rep}")

    # --- input DMAs, two parallel legs so wave-0 is never DMA-paced.
    # Weights ride the scalar queue in first-use order; xT (the bulky leg)
    # rides the gpsimd queue so the two legs transfer concurrently.
    for ki in range(8):
        nc.scalar.dma_start(wqk[:, ki, :], wqk_d[:, ki, :])
    nc.scalar.dma_start(bqk[:], bqk_d[:])
    nc.scalar.dma_start(tri[:], tri_d[:])
    for ki in range(8):
        nc.scalar.dma_start(wv[:, ki, :], wv_d[:, ki, :])
    for kc in range(4):
        nc.scalar.dma_start(wo[:, kc, :], wo_d[:, kc, :])
    # xT split across two DMA queues: wave0 is paced by this 4MB load,
    # and one HWDGE sustains only ~358GB/s (~11us); two queues halve the
    # ramp. Even/odd ki chunks alternate so ki-major wave0 consumption
    # matches the interleaved arrival order.
    for ki in range(8):
        (nc.gpsimd if ki % 2 == 0 else nc.sync).dma_start(
            xT[:, ki, :], xT_d[:, ki, :])

    # ones column for the PV-matmul row that accumulates l = sum P
    nc.vector.memset(v[:, :, :, 64], 1.0)

    # ---------------- stage-A building blocks ----------------

    def qk_finish(mch, t4, ps):
        """Bias (q-side only; k-bias is softmax-invariant -> dropped) folded
        into the PSUM->SBUF evacuation, one DVE pass."""
        if mch < 4:
            nc.vector.tensor_scalar_add(
                qkT[:, mch, t4 * 512:(t4 + 1) * 512], ps[:],
                bqk[:, mch:mch + 1])
        else:
            nc.vector.tensor_copy(qkT[:, mch, t4 * 512:(t4 + 1) * 512], ps[:])

    def qk_mms(mch, t4):
        """One qk projection group as 8 single-matmul thunks."""
        st = {}

        def mk(ki):
            def f():
                if ki == 0:
                    st["ps"] = mmps.tile([P, 512], F32, tag="mm",
                                         name=f"mmq{rep}_{mch}_{t4}")
                ps = st["ps"]
                nc.tensor.matmul(
                    ps[:],
                    lhsT=wqk[:, ki, mch * 128:(mch + 1) * 128],
                    rhs=xT[:, ki, t4 * 512:(t4 + 1) * 512],
                    start=(ki == 0), stop=(ki == 7),
                )
                if ki == 7:
                    qk_finish(mch, t4, ps)
            return f
        return [mk(k) for k in range(8)]

    def v_finish(mt, ps):
        psr = ps[:].rearrange("p (h d) -> p h d", h=HPG)
        nc.vector.tensor_copy(v[:, mt, :, 0:64], psr[:])

    def v_mms(mt):
        st = {}

        def mk(ki):
            def f():
                if ki == 0:
                    st["ps"] = mmps.tile([P, 512], F32, tag="mm",
                                         name=f"mmv{rep}_{mt}")
                ps = st["ps"]
                nc.tensor.matmul(
                    ps[:],
                    lhsT=xT[:, ki, mt * 128:(mt + 1) * 128],
                    rhs=wv[:, ki, :],
                    start=(ki == 0), stop=(ki == 7),
                )
                if ki == 7:
                    v_finish(mt, ps)
            return f
        return [mk(k) for k in range(8)]

    def op_mms(mt, n2):
        st = {}

        def mk(kc):
            def f():
                if kc == 0:
                    st["ps"] = mmps.tile([P, 512], F32, tag="mm",
                                         name=f"mmo{rep}_{mt}_{n2}")
                ps = st["ps"]
                nc.tensor.matmul(
                    ps[:],
                    lhsT=yT[:, kc, mt * 128:(mt + 1) * 128],
                    rhs=wo[:, kc, n2 * 512:(n2 + 1) * 512],
                    start=(kc == 0), stop=(kc == 3),
                )
                if kc == 3:
                    osb = outp.tile([P, 512], F32, tag="o",
                                    name=f"o{rep}_{mt}_{n2}")
                    nc.vector.tensor_copy(osb[:], ps[:])
                    _STORE_DMA(nc).dma_start(
                        out_d[mt * 128:(mt + 1) * 128,
                              n2 * 512:(n2 + 1) * 512],
                        osb[:])
            return f
        return [mk(k) for k in range(4)]

    def wave0():
        """qk groups (mch 0..7, t4=0) ki-major across all 8 PSUM banks so the
        PE starts as soon as the first xT/wqk chunks land, then v 0..3."""
        tiles = []
        banks = []   # (mch, ps_ap)
        for i in range(2):
            tiles.append(mmps.tile([P, 512], F32, tag="mm",
                                   name=f"w0mm{rep}_{i}"))
            banks.append(tiles[-1][:])
        for i in range(2):
            t2 = sps.tile([P, 1024], F32, tag="s", name=f"w0s{rep}_{i}")
            tiles.append(t2)
            banks.append(t2[:, 0:512])
            banks.append(t2[:, 512:1024])
        t2 = ops.tile([P, 1024], F32, tag="po", name=f"w0o{rep}")
        tiles.append(t2)
        banks.append(t2[:, 0:512])
        banks.append(t2[:, 512:1024])
        for ki in range(8):
            for mch in range(8):
                nc.tensor.matmul(
                    banks[mch],
                    lhsT=wqk[:, ki, mch * 128:(mch + 1) * 128],
                    rhs=xT[:, ki, 0:512],
                    start=(ki == 0), stop=(ki == 7),
                )
        for mch in range(8):
            qk_finish(mch, 0, banks[mch])
        for mt in range(4):
            for g in v_mms(mt):
                g()

    # ---------------- attention ----------------

    def attn_hp(qc, hp, pull=None):
        """S -> exp(+tri) -> PV over k-tiles for one head pair, then the
        normalize chain for both heads. The ki loop is software-pipelined:
        PV(ki) is emitted after S/exp(ki+1), and filler matmuls (pull) sit
        between S(ki) and PV(ki-1) so the PE never head-of-line blocks on
        exp while independent projection work is ready."""
        nkt = 4 * (qc + 1)
        po2 = ops.tile([P, 1024], F32, tag="po", name=f"po{rep}_{qc}_{hp}")

        def pv(pt2, n0, ki):
            for hh in range(2):
                nc.tensor.matmul(
                    po2[0:65, hh * 512 + n0:hh * 512 + 512],
                    lhsT=v[:, ki, 2 * hp + hh, :],
                    rhs=pt2[:, hh * 512 + n0:hh * 512 + 512],
                    start=(ki == 0), stop=(ki == nkt - 1),
                )

        prev = None
        for ki in range(nkt):
            j = ki - 4 * qc
            n0 = 128 * j if j >= 0 else 0
            s2 = sps.tile([P, 1024], F32, tag="s",
                          name=f"s{rep}_{qc}_{hp}_{ki}")
            for hh in range(2):
                pb = hh * 64
                nc.tensor.matmul(
                    s2[:, hh * 512 + n0:hh * 512 + 512],
                    lhsT=qkT[pb:pb + 64, 4 + hp, ki * 128:(ki + 1) * 128],
                    rhs=qkT[pb:pb + 64, hp, qc * 512 + n0:(qc + 1) * 512],
                    start=True, stop=True,
                )
            pt2 = ppool.tile([P, 1024], DT, tag="p",
                             name=f"p{rep}_{qc}_{hp}_{ki}")
            s2w = s2[:].rearrange("p (h w) -> p h w", h=2)
            pt2w = pt2[:].rearrange("p (h w) -> p h w", h=2)
            if _ABLATE_EXP:
                nc.scalar.activation(
                    pt2w[:, :, n0:n0 + 1], s2w[:, :, n0:n0 + 1], EXP)
            else:
                nc.scalar.activation(
                    pt2w[:, :, n0:512], s2w[:, :, n0:512], EXP)
            if j >= 0:
                for hh in range(2):
                    _TRI_ENGINE(nc).tensor_tensor(
                        pt2[:, hh * 512 + n0:hh * 512 + n0 + 128],
                        pt2[:, hh * 512 + n0:hh * 512 + n0 + 128],
                        tri[:], MUL)
            if pull is not None:
                pull()
            if prev is not None:
                pv(*prev)
            prev = (pt2, n0, ki)
        pv(*prev)
        # --- normalize both heads: O rows at partitions 0-63, l at 64 ---
        # One 65-partition copy evacuates PSUM (DVE time only counts the
        # free dim, so 65 partitions cost the same as 1); then
        # spread [1,1024]->[64,16] (DMA), reciprocal on 64 lanes, gather
        # back, partition-broadcast via 0-stride-source DMA. The direct
        # 1-partition reciprocal would be ~4.3us/call.
        ol = npool.tile([P, 1040], F32, tag="l", name=f"l{rep}_{qc}_{hp}")
        nc.scalar.copy(ol[0:65, 0:1024], po2[0:65, :])
        _CHAIN_DMA(nc).dma_start(
            ol[0:64, 1024:1040],
            ol[64:65, 0:1024].rearrange("o (p e) -> o p e", p=64))
        nc.vector.reciprocal(ol[0:64, 1024:1040], ol[0:64, 1024:1040])
        _CHAIN_DMA(nc).dma_start(ol[64:65, 0:1024], ol[0:64, 1024:1040])
        rb = npool.tile([64, 1024], F32, tag="rb", name=f"rb{rep}_{qc}_{hp}")
        _CHAIN_DMA(nc).dma_start(
            rb[0:64, :],
            ol[64:65, None, 0:1024].to_broadcast((1, 64, 1024)))
        nc.vector.tensor_tensor(
            yT[0:64, hp, qc * 512:(qc + 1) * 512],
            ol[0:64, 0:512], rb[0:64, 0:512], MUL)
        # DVE is partition-aligned; normalize head B at 0-63 then DMA-shift
        # the tile to partitions 64-127 of yT.
        tmp = npool.tile([64, 512], DT, tag="tmp", name=f"tmp{rep}_{qc}_{hp}")
        nc.vector.tensor_tensor(
            tmp[:], ol[0:64, 512:1024], rb[0:64, 512:1024], MUL)
        _STORE_DMA(nc).dma_start(
            yT[64:128, hp, qc * 512:(qc + 1) * 512], tmp[:])

    # ---------------- emission: fine-grained weave ----------------

    def weave(qc, queue):
        """attn chunk qc (4 head-pairs), pulling the filler micro-op queue
        evenly across its 4*nkt ki-iterations."""
        iters = 4 * (qc + 1) * 4
        n = len(queue)
        st = {"done": 0, "it": 0}

        def pull():
            st["it"] += 1
            tgt = min(n, (n * st["it"] + iters - 1) // iters)
            while st["done"] < tgt:
                queue[st["done"]]()
                st["done"] += 1
        for hp in range(4):
            attn_hp(qc, hp, pull)
        while st["done"] < n:   # safety drain (should be empty)
            queue[st["done"]]()
            st["done"] += 1

    def wave_queue(t4):
        """wave t4 as micro-ops, ordered so the groups attn(t4, hp0) needs
        first come first: q(hp0), k(hp0), the v tiles, then the rest."""
        gs = []
        gs += qk_mms(0, t4) + qk_mms(4, t4)
        for mt in range(4 * t4, 4 * t4 + 4):
            gs += v_mms(mt)
        for hp in range(1, 4):
            gs += qk_mms(hp, t4) + qk_mms(4 + hp, t4)
        return gs

    def op_queue(chunks):
        gs = []
        for qc in chunks:
            for mt in range(4 * qc, 4 * qc + 4):
                for n2 in range(2):
                    gs += op_mms(mt, n2)
        return gs

    wave0()
    weave(0, wave_queue(1))
    weave(1, wave_queue(2))
    weave(2, wave_queue(3))
    weave(3, op_queue([0, 1, 2]))
    for g in op_queue([3]):
        g()


def _ktiled(a, np_dtype):
    """[C_in, N] -> [128, C_in//128, N] (contraction partition-tiled)."""
    cin, n = a.shape
    return np.ascontiguousarray(
        a.reshape(cin // P, P, n).transpose(1, 0, 2)).astype(np_dtype)


def _make_in_maps(x, qkv_w, qkv_b, out_w, out_b):
    scale = 1.0 / np.sqrt(D)
    qw = qkv_w[0:C].reshape(H, D, C)
    kw = qkv_w[C:2 * C].reshape(H, D, C)
    vw = qkv_w[2 * C:3 * C].reshape(H, D, C)
    qb = qkv_b[0:C].reshape(H, D)

    tri = np.where(np.arange(P)[None, :] >= np.arange(P)[:, None],
                   np.float32(1), np.float32(0)).astype(BF16)

    xT_b = [_ktiled(np.ascontiguousarray(x[b].T), BF16) for b in range(B)]

    grp = []
    for g in range(2):
        hs = slice(g * HPG, (g + 1) * HPG)
        wqk_g = np.concatenate(
            [qw[hs].reshape(GC, C) * scale, kw[hs].reshape(GC, C)], 0)
        # k-side bias dropped (softmax-invariant); zero-fill its half
        bqk_g = np.concatenate(
            [qb[hs].reshape(GC) * scale, np.zeros(GC, np.float32)], 0)
        wv_g = vw[hs].reshape(GC, C)
        wo_g = out_w[:, g * GC:(g + 1) * GC]    # [1024, 512]
        grp.append({
            "wqk": _ktiled(np.ascontiguousarray(wqk_g.T), BF16),
            "bqk": np.ascontiguousarray(bqk_g.reshape(8, P).T).astype(np.float32),
            "wv": _ktiled(np.ascontiguousarray(wv_g.T), BF16),
            "wo": _ktiled(np.ascontiguousarray(wo_g.T), BF16),
        })

    in_maps = []
    for core in range(N_CORES):
        b, g = core // 2, core % 2
        in_maps.append({
            "xT": xT_b[b],
            "wqk": grp[g]["wqk"],
            "wv": grp[g]["wv"],
            "bqk": grp[g]["bqk"],
            "wo": grp[g]["wo"],
            "tri": tri,
        })
    return in_maps


def kernel(x, qkv_w, qkv_b, out_w, out_b):
    from concourse.bass_utils import run_bass_kernel_spmd

    x = np.asarray(x, np.float32)
    qkv_w = np.asarray(qkv_w, np.float32)
    qkv_b = np.asarray(qkv_b, np.float32)
    out_w = np.asarray(out_w, np.float32)
    out_b = np.asarray(out_b, np.float32)

    nc = _build_bass()
    in_maps = _make_in_maps(x, qkv_w, qkv_b, out_w, out_b)

    res = run_bass_kernel_spmd(
        nc, in_maps, core_ids=list(range(N_CORES)), trace=False)
    _BUILT["last_exec_time_ns"] = res.exec_time_ns
    _BUILT["in_maps"] = in_maps

    # host-side unshard: tensor-parallel all-reduce + all folded biases
    bias_full = out_b + out_w @ qkv_b[2 * C:3 * C]
    out = np.empty((B, T, C), np.float32)
    for b in range(B):
        out[b] = (np.asarray(res.results[2 * b]["out"])
                  + np.asarray(res.results[2 * b + 1]["out"])
                  + bias_full[None, :])
    return out
